# revision 19
# baseline (speedup 1.0000x reference)
"""CBOW (nn_CBOW_88991722373900) Trainium2 kernel.

Full-input contract: kernel(context_words[10,128000] f32, W_in[300,128000] f32,
W_out[128000,300] f32) -> softmax probabilities [128000] f32.

Strategy (8-way tensor/model parallel over the vocab dim V):
  - shard V into 8 chunks of 16000; each core holds its slice of both weight
    matrices (cast to bf16 on host - memory-bound problem, halves HBM traffic)
  - GEMM1: partial hidden[10,300] = ctx_shard^T-chunks (stationary) x
    W_inT-shard tiles (moving), accumulated in PSUM over 125 v-chunks
  - C-reduce + AllGather(300 floats) + rank-sum -> full hidden, transposed to
    n-on-partitions via PE transposes
  - GEMM2: logits[128,125] = W_outT 128-col blocks (stationary) x hidden col
    (moving), v mapped so partition p holds contiguous v = 125*p + b
  - softmax: exp on ScalarE (no max subtraction needed: |logit| << 1 for this
    problem's scales), local sum via ones-matmul, AllGather(1 float) for the
    global denominator, scale, DMA out
"""

import numpy as np
import ml_dtypes

import concourse.bass as bass
import concourse.mybir as mybir
from concourse import bass_utils as _bu
from concourse import tile
from concourse.bass_utils import run_bass_kernel_spmd
from concourse.vector_clock import ScopedClock, VectorClock

# (walrus --enable-ldw-opt is incompatible with bass-emitted LDWEIGHTS;
# leaving it at the pinned default.)

V = 128000
N = 300
C = 10
W = 8              # cores
VL = V // W        # 16000 vocab per core
NJ = VL // 128     # 125 v-chunks for GEMM1
NB = VL // 128     # 125 v-blocks for GEMM2
NCH = [(0, 128), (128, 128), (256, 44)]  # n-chunks
W2G = 16           # v-blocks per w2 SBUF tile group
NG2 = (NB + W2G - 1) // W2G  # 8 w2 column groups
W1J = 10           # v-chunks per w1 SBUF tile group
NG1 = (NJ + W1J - 1) // W1J  # 13 w1 groups

BF16 = mybir.dt.bfloat16
F32 = mybir.dt.float32
NP_BF16 = ml_dtypes.bfloat16


def _patched_drain_and_barrier(self, tick_clock, wait_clock):
    """Tail-drain waits split into 1-wait NOPs: this walrus build's CTRL
    instructions only encode a single sync wait."""
    vc = tick_clock.global_clock
    procs = [(p, vc[p]) for p in range(len(vc)) if vc[p] > 0]
    for i, (p, t) in enumerate(procs):
        pvc = VectorClock([0] * len(vc))
        pvc.require_at_least(p, t)
        nop_inst = self.nc.sync.nop(nofuse=True, hint=f"tail_wait_{i}")
        wait_clock.add_sem_waits(nop_inst.ins, ScopedClock({None: pvc}))
    self.nc.sync.drain()
    self.nc.all_engine_barrier(sem_only=True)
    assert self.sems is not None
    popped = self.nc._tile_sem_poison_stack.pop()
    assert popped is self._sem_poison
    self.nc.clear_and_free_semaphores(list(self.sems.allocated().values()))
    self.nc.all_engine_barrier(sem_only=True)


tile.TileContext._drain_and_barrier = _patched_drain_and_barrier


def _split_multi_waits(nc):
    """This walrus build encodes at most ONE sync wait per instruction. Hoist
    excess waits onto same-engine NoOps inserted immediately before."""
    import bass_rust

    ctr = [0]

    def make_nop(engine, wait):
        ctr[0] += 1
        nop = mybir.InstNoOp(name=f"I-wsplit{ctr[0]}", engine=engine)
        nop.bass_nofuse = True
        nop.sync_info = bass_rust.SyncInfo(on_wait=[wait], on_update=[])
        nc.register_instruction(nop, overwrite=True)
        return nop

    for bb in nc.main_func.blocks:
        out = []
        for ins in bb.instructions:
            si = ins.sync_info
            if si is not None and si.on_wait and len(si.on_wait) > 1:
                waits = list(si.on_wait)
                for w in waits[:-1]:
                    out.append(make_nop(ins.engine, w))
                ins.sync_info = bass_rust.SyncInfo(
                    on_wait=[waits[-1]], on_update=list(si.on_update)
                )
            out.append(ins)
        bb.instructions = out


def build_kernel():
    nc = bass.Bass()

    ctxp = nc.dram_tensor("ctxp", [128, NJ * C], BF16, kind="ExternalInput")
    # w1t packed partition-major on host: w1t[p, j*N + n] = W_in[n, v0+128j+p]
    w1t = nc.dram_tensor("w1t", [128, NJ * N], BF16, kind="ExternalInput")
    w2p = nc.dram_tensor("w2p", [N, VL], BF16, kind="ExternalInput")
    y_out = nc.dram_tensor("y", [128, NB], F32, kind="ExternalOutput")

    with tile.TileContext(nc) as tc:
        with (
            tc.tile_pool(name="const", bufs=1) as cpool,
            tc.tile_pool(name="psum", bufs=1, space="PSUM") as ppool,
            tc.tile_pool(name="dram", bufs=1, space="DRAM") as dpool,
        ):
            # ---- constants / inputs staged early ----
            ctx_sb = cpool.tile([128, NJ * C], BF16, tag="ctx")
            nc.gpsimd.dma_start(ctx_sb[:, :], ctxp[:, :])

            # warm the gpsimd instruction stream early so the collective
            # doorbell isn't its cold first op
            gp_warm = cpool.tile([1, 8], F32, tag="gpw")
            nc.gpsimd.memset(gp_warm[:, :], 0.0)

            ones80 = cpool.tile([W * C, 1], F32, tag="ones80")
            nc.vector.memset(ones80[:, :], 1.0)
            ones128 = cpool.tile([128, 1], F32, tag="ones128")
            nc.vector.memset(ones128[:, :], 1.0)
            ones_row = cpool.tile([1, 128], F32, tag="ones_row")
            nc.vector.memset(ones_row[:, :], 1.0)
            ident1 = cpool.tile([1, 1], F32, tag="ident1")
            nc.vector.memset(ident1[:, :], 1.0)

            # ---- w1 stream, alternating between the two HWDGE rings (SP and
            #      ACT) so all 16 SDMA engines stay fed; partition-major
            #      packed on host so each partition reads one contiguous run.
            #      First group kept small so GEMM1 starts early. ----
            w1_groups = [3, 7] + [10] * 11 + [5]  # sums to 125
            w1_sb = []
            j0 = 0
            for g, nj in enumerate(w1_groups):
                t = cpool.tile([128, nj * N], BF16, tag=f"w1_{g}")
                ring = nc.sync if g % 2 == 0 else nc.scalar
                ring.dma_start(t[:, :], w1t[:, j0 * N:(j0 + nj) * N])
                w1_sb.append((t, j0, nj))
                j0 += nj

            # w2 stream right behind w1, alternating rings; g-major order so
            # GEMM2's first v-blocks have all three n-chunk tiles early
            w2_sb = {}
            w2i = 0
            for g in range(NG2):
                b0 = g * W2G
                nb = min(W2G, NB - b0)
                for i3, (off, kk) in enumerate(NCH):
                    t = cpool.tile([kk, nb * 128], BF16, tag=f"w2_{i3}_{g}")
                    ring = nc.sync if w2i % 2 == 0 else nc.scalar
                    w2i += 1
                    ring.dma_start(
                        t[:, :], w2p[off:off + kk, b0 * 128:(b0 + nb) * 128]
                    )
                    w2_sb[(i3, g)] = t

            # ---- GEMM1: psum_h[c, n] += ctx_chunk^T x w1 tile ----
            psum_h = ppool.tile([C, N], F32, tag="ph")
            for t, j0g, nj in w1_sb:
                for jj in range(nj):
                    j = j0g + jj
                    nc.tensor.matmul(
                        psum_h[:, :],
                        ctx_sb[:, j * C:(j + 1) * C],
                        t[:, jj * N:(jj + 1) * N],
                        start=(j == 0),
                        stop=(j == NJ - 1),
                    )

            # ---- AllGather the [10, 300] partials (C-reduce folded into
            #      the post-gather ones-matmul) ----
            h10 = cpool.tile([C, N], F32, tag="h10")
            nc.vector.tensor_copy(h10[:, :], psum_h[:, :])
            cc_in = dpool.tile([C, N], F32, tag="cc_in")
            cc_out = dpool.tile([W * C, N], F32, tag="cc_out")
            nc.gpsimd.dma_start(cc_in[:, :], h10[:, :])
            nc.gpsimd.collective_compute(
                "AllGather",
                mybir.AluOpType.bypass,
                replica_groups=[list(range(W))],
                ins=[cc_in.opt()],
                outs=[cc_out.opt()],
            )
            hall = cpool.tile([W * C, N], F32, tag="hall")
            nc.gpsimd.dma_start(hall[:, :], cc_out[:, :])

            # ---- rank+context sum ----
            psum_hf = ppool.tile([1, N], F32, tag="phf")
            nc.tensor.matmul(psum_hf[:, :], ones80[:, :], hall[:, :])
            h_f32 = cpool.tile([1, N], F32, tag="hf32")
            nc.vector.tensor_copy(h_f32[:, :], psum_hf[:, :])

            # ---- transpose hidden to n-on-partitions [128, 3], scale 1/C,
            #      cast bf16 on the PSUM->SBUF copy ----
            psum_t = ppool.tile([128, 3], F32, tag="pt")
            for i3, (off, kk) in enumerate(NCH):
                nc.tensor.transpose(
                    psum_t[0:kk, i3:i3 + 1], h_f32[:, off:off + kk], ident1[:, :]
                )
            h_nt = cpool.tile([128, 3], BF16, tag="hnt")
            nc.vector.tensor_scalar_mul(h_nt[:, 0:2], psum_t[:, 0:2], 1.0 / C)
            nc.vector.tensor_scalar_mul(h_nt[0:44, 2:3], psum_t[0:44, 2:3], 1.0 / C)

            # ---- GEMM2: logits[p, b] = sum_n w2p[n, 128b+p] * h[n] ----
            psum_l = ppool.tile([128, NB], F32, tag="pl")
            for b in range(NB):
                g, bb = divmod(b, W2G)
                for i3, (off, kk) in enumerate(NCH):
                    nc.tensor.matmul(
                        psum_l[:, b:b + 1],
                        w2_sb[(i3, g)][:, bb * 128:(bb + 1) * 128],
                        h_nt[0:kk, i3:i3 + 1],
                        start=(i3 == 0),
                        stop=(i3 == 2),
                    )

            # ---- softmax (no max subtraction; |logits| << 1 here) ----
            e_sb = cpool.tile([128, NB], F32, tag="esb")
            esum = cpool.tile([128, 1], F32, tag="esum")
            nc.scalar.activation(
                e_sb[:, :],
                psum_l[:, :],
                mybir.ActivationFunctionType.Exp,
                accum_out=esum[:, :],
            )
            psum_s = ppool.tile([1, 1], F32, tag="ps")
            nc.tensor.matmul(psum_s[:, :], esum[:, :], ones128[:, :])
            ls = cpool.tile([1, 1], F32, tag="ls")
            nc.vector.tensor_copy(ls[:, :], psum_s[:, :])

            cc2_in = dpool.tile([1, 1], F32, tag="cc2_in")
            cc2_out = dpool.tile([1, W], F32, tag="cc2_out")
            nc.gpsimd.dma_start(cc2_in[:, :], ls[:, :])
            nc.gpsimd.collective_compute(
                "AllGather",
                mybir.AluOpType.bypass,
                replica_groups=[list(range(W))],
                ins=[cc2_in.opt()],
                outs=[cc2_out.opt()],
            )
            sall = cpool.tile([1, W], F32, tag="sall")
            nc.gpsimd.dma_start(sall[:, :], cc2_out[:, :])

            tsum = cpool.tile([1, 1], F32, tag="tsum")
            nc.vector.tensor_reduce(
                tsum[:, :], sall[:, :], mybir.AxisListType.X, mybir.AluOpType.add
            )
            rinv = cpool.tile([1, 1], F32, tag="rinv")
            nc.vector.reciprocal(rinv[:, :], tsum[:, :])

            # broadcast 1/sum to all partitions
            psum_b = ppool.tile([128, 1], F32, tag="pb")
            nc.tensor.matmul(psum_b[:, :], ones_row[:, :], rinv[:, :])
            rb = cpool.tile([128, 1], F32, tag="rb")
            nc.vector.tensor_copy(rb[:, :], psum_b[:, :])

            y_sb = cpool.tile([128, NB], F32, tag="ysb")
            nc.vector.tensor_scalar_mul(y_sb[:, :], e_sb[:, :], rb[:, :])
            nc.gpsimd.dma_start(y_out[:, :], y_sb[:, :])

    _split_multi_waits(nc)
    return nc


_NC_CACHE = None


def _get_nc():
    global _NC_CACHE
    if _NC_CACHE is None:
        _NC_CACHE = build_kernel()
    return _NC_CACHE


def _prep_inputs(context_words, W_in, W_out):
    """Host-side shard + layout prep (pure data movement + dtype cast)."""
    in_maps = []
    for r in range(W):
        v0 = r * VL
        ctx_s = np.asarray(context_words[:, v0:v0 + VL], dtype=NP_BF16)
        # ctxp[p, j*C + c] = ctx[c, 128j + p]
        ctxp = np.ascontiguousarray(
            ctx_s.reshape(C, NJ, 128).transpose(2, 1, 0).reshape(128, NJ * C)
        )
        # w1t[p, j*N + n] = W_in[n, v0 + 128j + p]  (partition-major pack)
        w1t = np.ascontiguousarray(
            W_in[:, v0:v0 + VL].T.astype(NP_BF16)
            .reshape(NJ, 128, N).transpose(1, 0, 2).reshape(128, NJ * N)
        )
        # w2p[n, 128b + p] = W_out[v0 + 125p + b, n]
        ws = np.asarray(W_out[v0:v0 + VL, :], dtype=NP_BF16)
        w2p = np.ascontiguousarray(
            ws.reshape(128, NB, N).transpose(2, 1, 0).reshape(N, VL)
        )
        in_maps.append({"ctxp": ctxp, "w1t": w1t, "w2p": w2p})
    return in_maps


def kernel(context_words, W_in, W_out):
    nc = _get_nc()
    in_maps = _prep_inputs(context_words, W_in, W_out)
    res = run_bass_kernel_spmd(nc, in_maps, list(range(W)))
    # y[p, b] on core r = prob[r*VL + 125*p + b]
    return np.concatenate(
        [np.asarray(res.results[r]["y"], dtype=np.float32).reshape(VL) for r in range(W)]
    )
